# revision 26
# baseline (speedup 1.0000x reference)
"""BiLSTM-CRF loss on 8 Trainium2 NeuronCores (pure data parallel over batch).

Strategy (per core, batch shard B=64):
  Phase 0: embedding gather (indirect DMA row gather) + PE transpose -> xT [65, B*L]
           (row 64 = ones, so the LSTM bias rides the x-matmul).
  Loop 1:  fwd and bwd LSTM run together, partition-stacked: fwd batch rows on
           partitions 0..63, bwd batch rows on 64..127.  All four gates use one
           sigmoid (tanh(x) = 2*sigmoid(2x)-1 folded into the weights; hidden
           state is tracked as h' = h/2 so h' = (sigmoid(2c)-0.5)*sigmoid(z_o)).
           Per-step transposed hidden states stored (bf16) for the logits pass.
  Loop 2:  logits^T chunks [18, 512] = Wd'^T hT (+ -1e4 * invalid-mask via a
           K=1 matmul) -> exp(logits+bd) -> g;  ghat = g * onehot(labels).
  Loop 3:  CRF forward recurrence + gold-path score as two parallel scaled
           exp-domain chains: state [19, 128] (19th row = "graveyard" that
           captures the terminal mass when a sequence's mask ends), one 19x19
           constant-stationary matmul + one DVE multiply per step, periodic
           per-column rescaling with log-scale accumulation.
  ll = path_score - log_norm, gathered to the host.
"""

import numpy as np
import ml_dtypes
from contextlib import ExitStack

import concourse.bacc as bacc
import concourse.bass as bass
import concourse.tile as tile
from concourse import mybir
from concourse.bass_utils import run_bass_kernel_spmd
from concourse.masks import make_identity

AF = mybir.ActivationFunctionType
ALU = mybir.AluOpType
F32 = mybir.dt.float32
BF16 = mybir.dt.bfloat16
I32 = mybir.dt.int32

EMB = 64
RNN = 128
K = 18
NCORES = 8
B = 64          # batch rows per core
L_FULL = 256
V_FULL = 30001
G = 4 * RNN     # 512 gate columns per direction


def build_program(L=L_FULL, V=V_FULL, w_rescale=12, dbg=False, stop_after="full"):
    NT = B * L
    nc = bacc.Bacc("TRN2", target_bir_lowering=False, debug=False)
    dbg_t = {}
    if dbg:
        dbg_t["dxT"] = nc.dram_tensor("dxT", [EMB + 1, NT], BF16, kind="ExternalOutput")
        dbg_t["dhj"] = nc.dram_tensor("dhj", [128, L * 128], BF16, kind="ExternalOutput")
        dbg_t["dgg"] = nc.dram_tensor("dgg", [K + 1, 2 * NT], BF16, kind="ExternalOutput")
        dbg_t["dast"] = nc.dram_tensor("dast", [K + 1, 2 * B], F32, kind="ExternalOutput")
        dbg_t["dls"] = nc.dram_tensor("dls", [1, 2 * B], F32, kind="ExternalOutput")
        dbg_t["dr1"] = nc.dram_tensor("dr1", [1, 2 * B], F32, kind="ExternalOutput")
        dbg_t["dz1"] = nc.dram_tensor("dz1", [1, 2 * B], F32, kind="ExternalOutput")
        dbg_t["da13"] = nc.dram_tensor("da13", [K + 1, 2 * B], F32, kind="ExternalOutput")

    emb = nc.dram_tensor("emb", [V, EMB], F32, kind="ExternalInput")
    xidx = nc.dram_tensor("xidx", [128, NT // 128], I32, kind="ExternalInput")
    wx = nc.dram_tensor("wx", [EMB + 1, 2 * G], BF16, kind="ExternalInput")
    wh = nc.dram_tensor("wh", [RNN, 2 * G], BF16, kind="ExternalInput")
    wd = nc.dram_tensor("wd", [2 * RNN, K], BF16, kind="ExternalInput")
    bdt = nc.dram_tensor("bdt", [K, 1], F32, kind="ExternalInput")
    tmat = nc.dram_tensor("tmat", [K, K], F32, kind="ExternalInput")
    ohd = nc.dram_tensor("ohd", [K, NT], BF16, kind="ExternalInput")
    pend = nc.dram_tensor("pend", [1, NT], BF16, kind="ExternalInput")
    grvd = nc.dram_tensor("grvd", [1, NT], BF16, kind="ExternalInput")
    llo = nc.dram_tensor("ll", [1, B], F32, kind="ExternalOutput")

    with tile.TileContext(nc) as tc, ExitStack() as ctx:
        const = ctx.enter_context(tc.tile_pool(name="const", bufs=1))
        persist = ctx.enter_context(tc.tile_pool(name="persist", bufs=1))

        ident = const.tile([128, 128], F32)
        make_identity(nc, ident[:])
        ident_bf = const.tile([128, 128], BF16)
        nc.vector.tensor_copy(ident_bf[:], ident[:])
        wx_sb = const.tile([EMB + 1, 2 * G], BF16)
        nc.sync.dma_start(wx_sb[:], wx[:])
        wh_sb = const.tile([RNN, 2 * G], BF16)
        nc.sync.dma_start(wh_sb[:], wh[:])
        wdf_sb = const.tile([RNN, K], BF16)
        nc.sync.dma_start(wdf_sb[:], wd[0:RNN, :])
        wdb_sb = const.tile([RNN, K], BF16)
        nc.sync.dma_start(wdb_sb[:], wd[RNN:2 * RNN, :])
        bdt_sb = const.tile([K, 1], F32)
        nc.sync.dma_start(bdt_sb[:], bdt[:])
        ones18 = const.tile([1, K], BF16)
        nc.vector.memset(ones18[:], 1.0)
        ones19 = const.tile([1, K + 1], F32)
        nc.vector.memset(ones19[:], 1.0)

        # Maug [19,33]: cols 0..17 = exp(T), col 18 = grave (ones: live-sum +
        # grave passthrough), col 32 = colsum readout (ones) placed at
        # partition 32 of the matmul output so PSUM row reads are aligned.
        MS = 33
        tm_sb = const.tile([K, K], F32)
        nc.sync.dma_start(tm_sb[:], tmat[:])
        maug = const.tile([K + 1, MS], F32)
        nc.vector.memset(maug[:], 0.0)
        nc.scalar.activation(maug[0:K, 0:K], tm_sb[:], AF.Exp)
        nc.vector.memset(maug[:, K:K + 1], 1.0)
        nc.vector.memset(maug[:, MS - 1:MS], 1.0)

        # persistent store: col t*128+b -> fwd hT for b<64, bwd hT for 64+b
        hjoint = persist.tile([128, L * 128], BF16)

        # ---------------- Phase 0 + Loop 1 (xT lives only here) -------------
        with tc.tile_pool(name="xT", bufs=1) as xpool:
            xT = xpool.tile([EMB + 1, NT], BF16)
            nc.vector.memset(xT[EMB:EMB + 1, :], 1.0)
            with (
                tc.tile_pool(name="gath", bufs=4) as gpool0,
                tc.tile_pool(name="gps", bufs=4, space="PSUM") as gps0,
                tc.tile_pool(name="idx", bufs=1) as ipool,
            ):
                idxt = ipool.tile([128, NT // 128], I32)
                nc.sync.dma_start(idxt[:], xidx[:])
                for k in range(NT // 128):
                    xg = gpool0.tile([128, EMB], F32, tag="xg")
                    nc.gpsimd.indirect_dma_start(
                        out=xg[:],
                        out_offset=None,
                        in_=emb[:],
                        in_offset=bass.IndirectOffsetOnAxis(
                            ap=idxt[:, k:k + 1], axis=0),
                    )
                    xps = gps0.tile([EMB, 128], F32, tag="xps")
                    nc.tensor.transpose(xps[:], xg[:], ident[:])
                    nc.vector.tensor_copy(
                        xT[0:EMB, k * 128:(k + 1) * 128], xps[:])

            if dbg:
                nc.sync.dma_start(dbg_t["dxT"][:], xT[:])
            with (
                tc.tile_pool(name="state", bufs=2) as spool,
                tc.tile_pool(name="zps", bufs=2, space="PSUM") as zpool,
                tc.tile_pool(name="tps", bufs=4, space="PSUM") as tpool,
                tc.tile_pool(name="gates", bufs=3) as gpool,
            ):
                # two independent chains: F(orward) and B(ackward), each B=64,
                # gate column order [g, i, f, o]
                hz = spool.tile([128, 2 * B], BF16, tag="h0")
                nc.vector.memset(hz[:], 0.0)
                h_prev = [hz[:, 0:B], hz[:, B:2 * B]]
                c_prev = []
                for ci in range(2):
                    cz = spool.tile([B, RNN], F32, tag=f"c{ci}")
                    nc.vector.memset(cz[:], 0.0)
                    c_prev.append(cz)
                for i in range(L):
                    ts_ = (i, L - 1 - i)
                    zs = []
                    for ci in range(2):
                        t = ts_[ci]
                        z = zpool.tile([B, G], F32, tag=f"z{ci}")
                        nc.tensor.matmul(z[:], lhsT=xT[:, t * B:(t + 1) * B],
                                         rhs=wx_sb[:, ci * G:(ci + 1) * G],
                                         start=True, stop=False)
                        nc.tensor.matmul(z[:], lhsT=h_prev[ci],
                                         rhs=wh_sb[:, ci * G:(ci + 1) * G],
                                         start=False, stop=True)
                        zs.append(z)
                    ss = []
                    for ci in range(2):
                        s = gpool.tile([B, G], BF16, tag=f"s{ci}")
                        nc.scalar.activation(s[:], zs[ci][:], AF.Sigmoid)
                        ss.append(s)
                    cs = []
                    for ci in range(2):
                        s = ss[ci]
                        v = gpool.tile([B, RNN], BF16, tag=f"v{ci}")
                        nc.vector.scalar_tensor_tensor(
                            v[:], in0=s[:, 0:RNN], scalar=-0.5,
                            in1=s[:, RNN:2 * RNN], op0=ALU.add, op1=ALU.mult)
                        p_ = gpool.tile([B, RNN], F32, tag=f"p{ci}")
                        nc.vector.tensor_tensor(p_[:], s[:, 2 * RNN:3 * RNN],
                                                c_prev[ci][:], op=ALU.mult)
                        c_new = spool.tile([B, RNN], F32, tag=f"c{ci}")
                        nc.vector.scalar_tensor_tensor(
                            c_new[:], in0=v[:], scalar=2.0, in1=p_[:],
                            op0=ALU.mult, op1=ALU.add)
                        cs.append(c_new)
                    ots = []
                    for ci in range(2):
                        ot_ps = tpool.tile([128, B], BF16, tag="tps")
                        nc.tensor.transpose(ot_ps[:], ss[ci][:, 3 * RNN:4 * RNN],
                                            ident_bf[0:B, 0:B])
                        ot = gpool.tile([128, B], BF16, tag=f"ot{ci}")
                        nc.vector.tensor_copy(ot[:], ot_ps[:])
                        ots.append(ot)
                    scts = []
                    for ci in range(2):
                        ct_ps = tpool.tile([128, B], F32, tag="tps")
                        nc.tensor.transpose(ct_ps[:], cs[ci][:], ident[0:B, 0:B])
                        sct = gpool.tile([128, B], BF16, tag=f"sct{ci}")
                        nc.scalar.activation(sct[:], ct_ps[:], AF.Sigmoid,
                                             scale=2.0)
                        scts.append(sct)
                    newh = []
                    for ci in range(2):
                        t = ts_[ci]
                        hsl = hjoint[:, t * 128 + ci * B:t * 128 + (ci + 1) * B]
                        nc.vector.scalar_tensor_tensor(
                            hsl, in0=scts[ci][:], scalar=-0.5, in1=ots[ci][:],
                            op0=ALU.add, op1=ALU.mult)
                        newh.append(hsl)
                    h_prev = newh
                    c_prev = cs
        if dbg:
            nc.sync.dma_start(dbg_t["dhj"][:], hjoint[:])
        if stop_after == "l1":
            nc.sync.dma_start(llo[:], hjoint[0:1, 0:2 * B].bitcast(F32))
        # ---------------- Loop 2: logits -> g, ghat ------------------------
        TCH = 8
        ggpool = ctx.enter_context(tc.tile_pool(name="gg", bufs=1))
        if stop_after == "l1":
            gg = None
        else:
            gg = ggpool.tile([K + 1, 2 * NT], BF16)   # [alpha-g | path-ghat]
            nc.sync.dma_start(gg[K:K + 1, 0:NT], grvd[:])
            nc.sync.dma_start(gg[K:K + 1, NT:2 * NT], grvd[:])
        with (
            tc.tile_pool(name="l2in", bufs=3) as l2pool,
            tc.tile_pool(name="lps", bufs=4, space="PSUM") as lpool,
        ):
            hj3 = hjoint[:].rearrange("p (t c) -> p t c", c=128)
            for q in range(L // TCH if stop_after != "l1" else 0):
                t0 = q * TCH
                cw = TCH * B
                ohc = l2pool.tile([K, cw], BF16, tag="ohc")
                nc.sync.dma_start(ohc[:], ohd[:, t0 * B:t0 * B + cw])
                penc = l2pool.tile([1, cw], BF16, tag="penc")
                nc.sync.dma_start(penc[:], pend[:, t0 * B:t0 * B + cw])
                lp = lpool.tile([K, cw], F32)
                nc.tensor.matmul(lp[:], lhsT=wdf_sb[:],
                                 rhs=hj3[:, t0:t0 + TCH, 0:B],
                                 start=True, stop=False)
                nc.tensor.matmul(lp[:], lhsT=wdb_sb[:],
                                 rhs=hj3[:, t0:t0 + TCH, B:2 * B],
                                 start=False, stop=False)
                nc.tensor.matmul(lp[:], lhsT=ones18[:], rhs=penc[:],
                                 start=False, stop=True)
                nc.scalar.activation(gg[0:K, t0 * B:t0 * B + cw], lp[:],
                                     AF.Exp, bias=bdt_sb[:])
                nc.vector.tensor_tensor(
                    gg[0:K, NT + t0 * B:NT + t0 * B + cw],
                    gg[0:K, t0 * B:t0 * B + cw],
                    ohc[:], op=ALU.mult)

        if dbg:
            nc.sync.dma_start(dbg_t["dgg"][:], gg[:])
        if stop_after == "l2":
            nc.sync.dma_start(llo[:], gg[0:1, 0:2 * B].bitcast(F32))
        # ---------------- Loop 3: CRF chains (two interleaved) --------------
        do_l3 = stop_after == "full"
        if do_l3:
          with (
            tc.tile_pool(name="crf", bufs=3) as cpool,
            tc.tile_pool(name="crfps", bufs=2, space="PSUM") as cps,
          ):
            asts = []
            for ci in range(2):
                a0 = cpool.tile([K + 1, B], F32, tag=f"ast{ci}")
                nc.vector.memset(a0[:], 0.0)
                nc.vector.tensor_copy(a0[0:K, :], gg[0:K, ci * NT:ci * NT + B])
                asts.append(a0)
            lss = []
            for ci in range(2):
                l0 = cpool.tile([1, B], F32, tag=f"ls{ci}")
                nc.vector.memset(l0[:], 0.0)
                lss.append(l0)
            for t in range(1, L):
                pas = []
                for ci in range(2):
                    pa = cps.tile([MS, B], F32, tag=f"pa{ci}")
                    nc.tensor.matmul(pa[:], lhsT=maug[:], rhs=asts[ci][:],
                                     start=True, stop=True)
                    pas.append(pa)
                resc = (t % w_rescale == 0)
                news = []
                for ci in range(2):
                    a_new = cpool.tile([K + 1, B], F32, tag=f"ast{ci}")
                    gsl = gg[0:K + 1, ci * NT + t * B:ci * NT + (t + 1) * B]
                    if resc:
                        zrow = cpool.tile([1, B], F32, tag=f"zr{ci}")
                        nc.vector.tensor_copy(zrow[:], pas[ci][MS - 1:MS, :])
                        r = cpool.tile([1, B], F32, tag=f"r{ci}")
                        nc.vector.reciprocal(r[:], zrow[:])
                        pr = cps.tile([K + 1, B], F32, tag=f"pr{ci}")
                        nc.tensor.matmul(pr[:], lhsT=ones19[:], rhs=r[:],
                                         start=True, stop=True)
                        lnr = cpool.tile([1, B], F32, tag=f"lnr{ci}")
                        nc.scalar.activation(lnr[:], r[:], AF.Ln)
                        ls_new = cpool.tile([1, B], F32, tag=f"ls{ci}")
                        nc.vector.tensor_tensor(ls_new[:], lss[ci][:], lnr[:],
                                                op=ALU.subtract)
                        lss[ci] = ls_new
                        atmp = cpool.tile([K + 1, B], F32, tag=f"atmp{ci}")
                        nc.vector.tensor_tensor(atmp[:], pas[ci][0:K + 1, :],
                                                gsl, op=ALU.mult)
                        nc.vector.tensor_tensor(a_new[:], atmp[:], pr[:],
                                                op=ALU.mult)
                    else:
                        nc.vector.tensor_tensor(a_new[:], pas[ci][0:K + 1, :],
                                                gsl, op=ALU.mult)
                    news.append(a_new)
                asts = news
            tots = []
            for ci in range(2):
                pf = cps.tile([MS, B], F32, tag=f"pa{ci}")
                nc.tensor.matmul(pf[:], lhsT=maug[:], rhs=asts[ci][:],
                                 start=True, stop=True)
                lnt = cpool.tile([1, B], F32, tag=f"lnt{ci}")
                nc.scalar.activation(lnt[:], pf[MS - 1:MS, :], AF.Ln)
                tot = cpool.tile([1, B], F32, tag=f"tot{ci}")
                nc.vector.tensor_tensor(tot[:], lnt[:], lss[ci][:], op=ALU.add)
                tots.append(tot)
            ll_sb = cpool.tile([1, B], F32, tag="ll")
            nc.vector.tensor_tensor(ll_sb[:], tots[1][:], tots[0][:],
                                    op=ALU.subtract)
            nc.sync.dma_start(llo[:], ll_sb[:])
            if dbg:
                nc.sync.dma_start(dbg_t["dast"][:, 0:B], asts[0][:])
                nc.sync.dma_start(dbg_t["dast"][:, B:2 * B], asts[1][:])
                nc.sync.dma_start(dbg_t["dls"][:, 0:B], lss[0][:])
                nc.sync.dma_start(dbg_t["dls"][:, B:2 * B], lss[1][:])

    nc.compile()
    return nc


# ---------------------------------------------------------------------------
# host side
# ---------------------------------------------------------------------------

def _pack_dir(Wx, Wh, b):
    i, f, g, o = np.split(np.asarray(Wx, np.float32), 4, axis=1)
    wxp = np.concatenate([2.0 * g, i, f, o], axis=1)
    i, f, g, o = np.split(np.asarray(Wh, np.float32), 4, axis=1)
    whp = np.concatenate([4.0 * g, 2.0 * i, 2.0 * f, 2.0 * o], axis=1)
    bi, bf_, bg, bo = np.split(np.asarray(b, np.float32), 4)
    bp = np.concatenate([2.0 * bg, bi, bf_, bo])
    return np.concatenate([wxp, bp[None, :]], axis=0), whp


def make_in_maps(inputs, labels, E, Wx_f, Wh_f, b_f, Wx_b, Wh_b, b_b, Wd, bd, T,
                 L=L_FULL):
    NT = B * L
    bf16 = ml_dtypes.bfloat16
    wxf, whf = _pack_dir(Wx_f, Wh_f, b_f)
    wxb, whb = _pack_dir(Wx_b, Wh_b, b_b)
    wx = np.concatenate([wxf, wxb], axis=1).astype(bf16)
    wh = np.concatenate([whf, whb], axis=1).astype(bf16)
    wd = (2.0 * np.asarray(Wd, np.float32)).astype(bf16)
    bdt = np.asarray(bd, np.float32).reshape(K, 1)
    tmat = np.asarray(T, np.float32)
    emb = np.ascontiguousarray(np.asarray(E, np.float32))
    tok = np.asarray(inputs).astype(np.int32)
    lab = np.asarray(labels).astype(np.int32)

    in_maps = []
    for c in range(NCORES):
        tk = tok[c * B:(c + 1) * B]          # [B, L]
        lb = lab[c * B:(c + 1) * B]
        ids = np.ascontiguousarray(tk.T).reshape(-1)          # t-major [NT]
        xidx = np.ascontiguousarray(ids.reshape(NT // 128, 128).T).astype(np.int32)
        labt = np.ascontiguousarray(lb.T).reshape(-1)
        oh = (labt[None, :] == np.arange(K, dtype=np.int64)[:, None])
        lens = (lb != 0).sum(axis=1)                          # [B]
        apf = (np.arange(L)[None, :] >= lens[:, None])        # [B, L] invalid
        apt = np.ascontiguousarray(apf.T).reshape(-1).astype(np.float32)
        pen = (-10000.0 * apt)[None, :]
        pen[0, 0:B] = 0.0
        pen = pen.astype(bf16)
        in_maps.append(dict(
            emb=emb, xidx=xidx, wx=wx, wh=wh, wd=wd, bdt=bdt, tmat=tmat,
            ohd=oh.astype(bf16), pend=pen, grvd=apt[None, :].astype(bf16),
        ))
    return in_maps


_PROG = None


def _get_prog():
    global _PROG
    if _PROG is None:
        _PROG = build_program()
    return _PROG


def kernel(inputs, labels, E, Wx_f, Wh_f, b_f, Wx_b, Wh_b, b_b, Wd, bd, T):
    nc = _get_prog()
    in_maps = make_in_maps(inputs, labels, E, Wx_f, Wh_f, b_f,
                           Wx_b, Wh_b, b_b, Wd, bd, T)
    res = run_bass_kernel_spmd(nc, in_maps, core_ids=list(range(NCORES)))
    ll = np.concatenate([res.results[c]["ll"].reshape(B) for c in range(NCORES)])
    return ll.astype(np.float32), np.asarray(T, np.float32)


# numpy mini-reference (float64) for testing at arbitrary L/V ----------------

def ref_numpy(inputs, labels, E, Wx_f, Wh_f, b_f, Wx_b, Wh_b, b_b, Wd, bd, T):
    f = np.float64
    tok = np.asarray(inputs); lab = np.asarray(labels)
    E = np.asarray(E, f); T = np.asarray(T, f)
    Bf, Lf = tok.shape

    def sig(x):
        return 1.0 / (1.0 + np.exp(-x))

    def lstm(x, Wx, Wh, b, reverse):
        Wx = np.asarray(Wx, f); Wh = np.asarray(Wh, f); b = np.asarray(b, f)
        h = np.zeros((Bf, RNN), f); c = np.zeros((Bf, RNN), f)
        hs = np.zeros((Lf, Bf, RNN), f)
        order = range(Lf - 1, -1, -1) if reverse else range(Lf)
        for t in order:
            z = x[t] @ Wx + h @ Wh + b
            i, fg, g, o = np.split(z, 4, axis=1)
            c = sig(fg) * c + sig(i) * np.tanh(g)
            h = sig(o) * np.tanh(c)
            hs[t] = h
        return hs

    x = E[tok].transpose(1, 0, 2)             # [L, B, E]
    hf = lstm(x, Wx_f, Wh_f, b_f, False)
    hb = lstm(x, Wx_b, Wh_b, b_b, True)
    h = np.concatenate([hf, hb], axis=2)      # [L, B, 2R]
    logits = h.transpose(1, 0, 2) @ np.asarray(Wd, f) + np.asarray(bd, f)
    lens = (lab != 0).sum(axis=1)
    pos = np.arange(Lf)[None, :] < lens[:, None]
    unary = np.take_along_axis(logits, lab[..., None], axis=2)[..., 0]
    unary = (unary * pos).sum(axis=1)
    binary = (T[lab[:, :-1], lab[:, 1:]] * pos[:, 1:]).sum(axis=1)
    alpha = logits[:, 0, :].copy()
    for t in range(1, Lf):
        new = np.log(np.exp(alpha[:, :, None] - alpha.max(1)[:, None, None]
                            + T[None]).sum(axis=1)) \
            + alpha.max(1)[:, None] + logits[:, t, :]
        alpha = np.where(pos[:, t][:, None], new, alpha)
    mx = alpha.max(1)
    log_norm = np.log(np.exp(alpha - mx[:, None]).sum(1)) + mx
    return unary + binary - log_norm


# revision 30
# speedup vs baseline: 1.0181x; 1.0181x over previous
"""BiLSTM-CRF loss on 8 Trainium2 NeuronCores (pure data parallel over batch).

Strategy (per core, batch shard B=64):
  Phase 0: embedding gather (indirect DMA row gather) + PE transpose -> xT [65, B*L]
           (row 64 = ones, so the LSTM bias rides the x-matmul).
  Loop 1:  fwd and bwd LSTM as two independent chains that pipeline against
           each other.  All four gates use one sigmoid per chain
           (tanh(x) = 2*sigmoid(2x)-1 folded into the weights; hidden state is
           tracked as h' = h/2 so h' = (sigmoid(2c)-0.5)*sigmoid(z_o)); the
           cell c is transposed on the PE and sigmoid(2c^T) reads the PSUM
           directly, writing the transposed bf16 state straight into the
           per-step hidden store used by the logits pass.
  Loop 2:  logits^T chunks [18, 512] = Wd'^T hT (+ -1e4 * invalid-mask via a
           K=1 matmul) -> exp(logits+bd) -> g;  ghat = g * onehot(labels).
  Loop 3:  CRF forward recurrence + gold-path score as two parallel scaled
           exp-domain chains: state [19, 128] (19th row = "graveyard" that
           captures the terminal mass when a sequence's mask ends), one 19x19
           constant-stationary matmul + one DVE multiply per step, periodic
           per-column rescaling (every 12 steps: keeps within-window growth
           under fp32 range AND the final pre-Ln colsum under the ScalarE Ln
           domain limit of 2^64) with log-scale accumulation.
  ll = path_score - log_norm, gathered to the host.
"""

import numpy as np
import ml_dtypes
from contextlib import ExitStack

import concourse.bacc as bacc
import concourse.bass as bass
import concourse.tile as tile
from concourse import mybir
from concourse.bass_utils import run_bass_kernel_spmd
from concourse.masks import make_identity

AF = mybir.ActivationFunctionType
ALU = mybir.AluOpType
F32 = mybir.dt.float32
BF16 = mybir.dt.bfloat16
I32 = mybir.dt.int32

EMB = 64
RNN = 128
K = 18
NCORES = 8
B = 64          # batch rows per core
L_FULL = 256
V_FULL = 30001
G = 4 * RNN     # 512 gate columns per direction


def build_program(L=L_FULL, V=V_FULL, w_rescale=12, dbg=False, stop_after="full"):
    NT = B * L
    nc = bacc.Bacc("TRN2", target_bir_lowering=False, debug=False)
    dbg_t = {}
    if dbg:
        dbg_t["dxT"] = nc.dram_tensor("dxT", [EMB + 1, NT], BF16, kind="ExternalOutput")
        dbg_t["dhj"] = nc.dram_tensor("dhj", [128, L * 128], BF16, kind="ExternalOutput")
        dbg_t["dgg"] = nc.dram_tensor("dgg", [K + 1, 2 * NT], BF16, kind="ExternalOutput")
        dbg_t["dast"] = nc.dram_tensor("dast", [K + 1, 2 * B], F32, kind="ExternalOutput")
        dbg_t["dls"] = nc.dram_tensor("dls", [1, 2 * B], F32, kind="ExternalOutput")
        dbg_t["dr1"] = nc.dram_tensor("dr1", [1, 2 * B], F32, kind="ExternalOutput")
        dbg_t["dz1"] = nc.dram_tensor("dz1", [1, 2 * B], F32, kind="ExternalOutput")
        dbg_t["da13"] = nc.dram_tensor("da13", [K + 1, 2 * B], F32, kind="ExternalOutput")

    emb = nc.dram_tensor("emb", [V, EMB], F32, kind="ExternalInput")
    xidx = nc.dram_tensor("xidx", [128, NT // 128], I32, kind="ExternalInput")
    wx = nc.dram_tensor("wx", [EMB + 1, 2 * G], BF16, kind="ExternalInput")
    wh = nc.dram_tensor("wh", [RNN, 2 * G], BF16, kind="ExternalInput")
    wd = nc.dram_tensor("wd", [2 * RNN, K], BF16, kind="ExternalInput")
    bdt = nc.dram_tensor("bdt", [K, 1], F32, kind="ExternalInput")
    tmat = nc.dram_tensor("tmat", [K, K], F32, kind="ExternalInput")
    ohd = nc.dram_tensor("ohd", [K, NT], BF16, kind="ExternalInput")
    pend = nc.dram_tensor("pend", [1, NT], BF16, kind="ExternalInput")
    grvd = nc.dram_tensor("grvd", [1, NT], BF16, kind="ExternalInput")
    llo = nc.dram_tensor("ll", [1, B], F32, kind="ExternalOutput")

    with tile.TileContext(nc) as tc, ExitStack() as ctx:
        const = ctx.enter_context(tc.tile_pool(name="const", bufs=1))
        persist = ctx.enter_context(tc.tile_pool(name="persist", bufs=1))

        ident = const.tile([128, 128], F32)
        make_identity(nc, ident[:])
        ident_bf = const.tile([128, 128], BF16)
        nc.vector.tensor_copy(ident_bf[:], ident[:])
        wx_sb = const.tile([EMB + 1, 2 * G], BF16)
        nc.sync.dma_start(wx_sb[:], wx[:])
        wh_sb = const.tile([RNN, 2 * G], BF16)
        nc.sync.dma_start(wh_sb[:], wh[:])
        wdf_sb = const.tile([RNN, K], BF16)
        nc.sync.dma_start(wdf_sb[:], wd[0:RNN, :])
        wdb_sb = const.tile([RNN, K], BF16)
        nc.sync.dma_start(wdb_sb[:], wd[RNN:2 * RNN, :])
        bdt_sb = const.tile([K, 1], F32)
        nc.sync.dma_start(bdt_sb[:], bdt[:])
        ones18 = const.tile([1, K], BF16)
        nc.vector.memset(ones18[:], 1.0)
        ones19 = const.tile([1, K + 1], F32)
        nc.vector.memset(ones19[:], 1.0)

        # Maug [19,33]: cols 0..17 = exp(T), col 18 = grave (ones: live-sum +
        # grave passthrough), col 32 = colsum readout (ones) placed at
        # partition 32 of the matmul output so PSUM row reads are aligned.
        MS = 33
        tm_sb = const.tile([K, K], F32)
        nc.sync.dma_start(tm_sb[:], tmat[:])
        maug = const.tile([K + 1, MS], F32)
        nc.vector.memset(maug[:], 0.0)
        nc.scalar.activation(maug[0:K, 0:K], tm_sb[:], AF.Exp)
        nc.vector.memset(maug[:, K:K + 1], 1.0)
        nc.vector.memset(maug[:, MS - 1:MS], 1.0)

        # persistent store: col t*128+b -> fwd hT for b<64, bwd hT for 64+b
        hjoint = persist.tile([128, L * 128], BF16)

        # ---------------- Phase 0 + Loop 1 (xT lives only here) -------------
        with tc.tile_pool(name="xT", bufs=1) as xpool:
            xT = xpool.tile([EMB + 1, NT], BF16)
            nc.vector.memset(xT[EMB:EMB + 1, :], 1.0)
            with (
                tc.tile_pool(name="gath", bufs=6) as gpool0,
                tc.tile_pool(name="idx", bufs=1) as ipool,
                tc.tile_pool(name="state", bufs=2) as spool,
                tc.tile_pool(name="zps", bufs=2, space="PSUM") as zpool,
                tc.tile_pool(name="tps", bufs=4, space="PSUM") as tpool,
                tc.tile_pool(name="gates", bufs=3) as gpool,
            ):
                idxt = ipool.tile([128, NT // 128], I32)
                nc.sync.dma_start(idxt[:], xidx[:])
                NK = NT // 128

                def emit_xtile(k):
                    xg = gpool0.tile([128, EMB], F32, tag="xg")
                    nc.gpsimd.indirect_dma_start(
                        out=xg[:],
                        out_offset=None,
                        in_=emb[:],
                        in_offset=bass.IndirectOffsetOnAxis(
                            ap=idxt[:, k:k + 1], axis=0),
                    )
                    xps = tpool.tile([EMB, 128], F32, tag="tps")
                    nc.tensor.transpose(xps[:], xg[:], ident[:])
                    nc.vector.tensor_copy(
                        xT[0:EMB, k * 128:(k + 1) * 128], xps[:])

                def emit_pair(m):  # gather tiles first needed at iter m (even)
                    emit_xtile(m // 2)
                    if NK - 1 - m // 2 != m // 2:
                        emit_xtile(NK - 1 - m // 2)

                PD = 6
                for m in range(0, PD, 2):
                    if m <= 2 * (NK - 1):
                        emit_pair(m)
                # two independent chains: F(orward) and B(ackward), each B=64,
                # gate column order [g, i, f, o]
                hz = spool.tile([128, 2 * B], BF16, tag="h0")
                nc.vector.memset(hz[:], 0.0)
                h_prev = [hz[:, 0:B], hz[:, B:2 * B]]
                c_prev = []
                for ci in range(2):
                    cz = spool.tile([B, RNN], F32, tag=f"c{ci}")
                    nc.vector.memset(cz[:], 0.0)
                    c_prev.append(cz)
                for i in range(L):
                    if (i + PD) % 2 == 0 and i + PD <= 2 * (NK - 1):
                        emit_pair(i + PD)
                    ts_ = (i, L - 1 - i)
                    zs = []
                    for ci in range(2):
                        t = ts_[ci]
                        z = zpool.tile([B, G], F32, tag=f"z{ci}")
                        nc.tensor.matmul(z[:], lhsT=xT[:, t * B:(t + 1) * B],
                                         rhs=wx_sb[:, ci * G:(ci + 1) * G],
                                         start=True, stop=False)
                        nc.tensor.matmul(z[:], lhsT=h_prev[ci],
                                         rhs=wh_sb[:, ci * G:(ci + 1) * G],
                                         start=False, stop=True)
                        zs.append(z)
                    ss = []
                    for ci in range(2):
                        s = gpool.tile([B, G], BF16, tag=f"s{ci}")
                        nc.scalar.activation(s[:], zs[ci][:], AF.Sigmoid)
                        ss.append(s)
                    cs = []
                    for ci in range(2):
                        s = ss[ci]
                        v = gpool.tile([B, RNN], BF16, tag=f"v{ci}")
                        nc.vector.scalar_tensor_tensor(
                            v[:], in0=s[:, 0:RNN], scalar=-0.5,
                            in1=s[:, RNN:2 * RNN], op0=ALU.add, op1=ALU.mult)
                        p_ = gpool.tile([B, RNN], F32, tag=f"p{ci}")
                        nc.vector.tensor_tensor(p_[:], s[:, 2 * RNN:3 * RNN],
                                                c_prev[ci][:], op=ALU.mult)
                        c_new = spool.tile([B, RNN], F32, tag=f"c{ci}")
                        nc.vector.scalar_tensor_tensor(
                            c_new[:], in0=v[:], scalar=2.0, in1=p_[:],
                            op0=ALU.mult, op1=ALU.add)
                        cs.append(c_new)
                    ots = []
                    for ci in range(2):
                        ot_ps = tpool.tile([128, B], BF16, tag="tps")
                        nc.tensor.transpose(ot_ps[:], ss[ci][:, 3 * RNN:4 * RNN],
                                            ident_bf[0:B, 0:B])
                        ot = gpool.tile([128, B], BF16, tag=f"ot{ci}")
                        nc.vector.tensor_copy(ot[:], ot_ps[:])
                        ots.append(ot)
                    scts = []
                    for ci in range(2):
                        ct_ps = tpool.tile([128, B], F32, tag="tps")
                        nc.tensor.transpose(ct_ps[:], cs[ci][:], ident[0:B, 0:B])
                        sct = gpool.tile([128, B], BF16, tag=f"sct{ci}")
                        nc.scalar.activation(sct[:], ct_ps[:], AF.Sigmoid,
                                             scale=2.0)
                        scts.append(sct)
                    newh = []
                    for ci in range(2):
                        t = ts_[ci]
                        hsl = hjoint[:, t * 128 + ci * B:t * 128 + (ci + 1) * B]
                        nc.vector.scalar_tensor_tensor(
                            hsl, in0=scts[ci][:], scalar=-0.5, in1=ots[ci][:],
                            op0=ALU.add, op1=ALU.mult)
                        newh.append(hsl)
                    h_prev = newh
                    c_prev = cs
        if dbg:
            nc.sync.dma_start(dbg_t["dhj"][:], hjoint[:])
        if stop_after == "l1":
            nc.sync.dma_start(llo[:], hjoint[0:1, 0:2 * B].bitcast(F32))
        # ---------------- Loop 2: logits -> g, ghat ------------------------
        TCH = 8
        ggpool = ctx.enter_context(tc.tile_pool(name="gg", bufs=1))
        if stop_after == "l1":
            gg = None
        else:
            gg = ggpool.tile([K + 1, 2 * NT], BF16)   # [alpha-g | path-ghat]
            nc.sync.dma_start(gg[K:K + 1, 0:NT], grvd[:])
            nc.sync.dma_start(gg[K:K + 1, NT:2 * NT], grvd[:])
        with (
            tc.tile_pool(name="l2in", bufs=3) as l2pool,
            tc.tile_pool(name="lps", bufs=4, space="PSUM") as lpool,
        ):
            hj3 = hjoint[:].rearrange("p (t c) -> p t c", c=128)
            for q in range(L // TCH if stop_after != "l1" else 0):
                t0 = q * TCH
                cw = TCH * B
                ohc = l2pool.tile([K, cw], BF16, tag="ohc")
                nc.sync.dma_start(ohc[:], ohd[:, t0 * B:t0 * B + cw])
                penc = l2pool.tile([1, cw], BF16, tag="penc")
                nc.sync.dma_start(penc[:], pend[:, t0 * B:t0 * B + cw])
                lp = lpool.tile([K, cw], F32)
                nc.tensor.matmul(lp[:], lhsT=wdf_sb[:],
                                 rhs=hj3[:, t0:t0 + TCH, 0:B],
                                 start=True, stop=False)
                nc.tensor.matmul(lp[:], lhsT=wdb_sb[:],
                                 rhs=hj3[:, t0:t0 + TCH, B:2 * B],
                                 start=False, stop=False)
                nc.tensor.matmul(lp[:], lhsT=ones18[:], rhs=penc[:],
                                 start=False, stop=True)
                nc.scalar.activation(gg[0:K, t0 * B:t0 * B + cw], lp[:],
                                     AF.Exp, bias=bdt_sb[:])
                nc.vector.tensor_tensor(
                    gg[0:K, NT + t0 * B:NT + t0 * B + cw],
                    gg[0:K, t0 * B:t0 * B + cw],
                    ohc[:], op=ALU.mult)

        if dbg:
            nc.sync.dma_start(dbg_t["dgg"][:], gg[:])
        if stop_after == "l2":
            nc.sync.dma_start(llo[:], gg[0:1, 0:2 * B].bitcast(F32))
        # ---------------- Loop 3: CRF chains (two interleaved) --------------
        do_l3 = stop_after == "full"
        if do_l3:
          with (
            tc.tile_pool(name="crf", bufs=3) as cpool,
            tc.tile_pool(name="crfps", bufs=2, space="PSUM") as cps,
          ):
            asts = []
            for ci in range(2):
                a0 = cpool.tile([K + 1, B], F32, tag=f"ast{ci}")
                nc.vector.memset(a0[:], 0.0)
                nc.vector.tensor_copy(a0[0:K, :], gg[0:K, ci * NT:ci * NT + B])
                asts.append(a0)
            lss = []
            for ci in range(2):
                l0 = cpool.tile([1, B], F32, tag=f"ls{ci}")
                nc.vector.memset(l0[:], 0.0)
                lss.append(l0)
            for t in range(1, L):
                pas = []
                for ci in range(2):
                    pa = cps.tile([MS, B], F32, tag=f"pa{ci}")
                    nc.tensor.matmul(pa[:], lhsT=maug[:], rhs=asts[ci][:],
                                     start=True, stop=True)
                    pas.append(pa)
                resc = (t % w_rescale == 0)
                news = []
                for ci in range(2):
                    a_new = cpool.tile([K + 1, B], F32, tag=f"ast{ci}")
                    gsl = gg[0:K + 1, ci * NT + t * B:ci * NT + (t + 1) * B]
                    if resc:
                        zrow = cpool.tile([1, B], F32, tag=f"zr{ci}")
                        nc.vector.tensor_copy(zrow[:], pas[ci][MS - 1:MS, :])
                        r = cpool.tile([1, B], F32, tag=f"r{ci}")
                        nc.vector.reciprocal(r[:], zrow[:])
                        pr = cps.tile([K + 1, B], F32, tag=f"pr{ci}")
                        nc.tensor.matmul(pr[:], lhsT=ones19[:], rhs=r[:],
                                         start=True, stop=True)
                        lnr = cpool.tile([1, B], F32, tag=f"lnr{ci}")
                        nc.scalar.activation(lnr[:], r[:], AF.Ln)
                        ls_new = cpool.tile([1, B], F32, tag=f"ls{ci}")
                        nc.vector.tensor_tensor(ls_new[:], lss[ci][:], lnr[:],
                                                op=ALU.subtract)
                        lss[ci] = ls_new
                        atmp = cpool.tile([K + 1, B], F32, tag=f"atmp{ci}")
                        nc.vector.tensor_tensor(atmp[:], pas[ci][0:K + 1, :],
                                                gsl, op=ALU.mult)
                        nc.vector.tensor_tensor(a_new[:], atmp[:], pr[:],
                                                op=ALU.mult)
                    else:
                        nc.vector.tensor_tensor(a_new[:], pas[ci][0:K + 1, :],
                                                gsl, op=ALU.mult)
                    news.append(a_new)
                asts = news
            tots = []
            for ci in range(2):
                pf = cps.tile([MS, B], F32, tag=f"pa{ci}")
                nc.tensor.matmul(pf[:], lhsT=maug[:], rhs=asts[ci][:],
                                 start=True, stop=True)
                lnt = cpool.tile([1, B], F32, tag=f"lnt{ci}")
                nc.scalar.activation(lnt[:], pf[MS - 1:MS, :], AF.Ln)
                tot = cpool.tile([1, B], F32, tag=f"tot{ci}")
                nc.vector.tensor_tensor(tot[:], lnt[:], lss[ci][:], op=ALU.add)
                tots.append(tot)
            ll_sb = cpool.tile([1, B], F32, tag="ll")
            nc.vector.tensor_tensor(ll_sb[:], tots[1][:], tots[0][:],
                                    op=ALU.subtract)
            nc.sync.dma_start(llo[:], ll_sb[:])
            if dbg:
                nc.sync.dma_start(dbg_t["dast"][:, 0:B], asts[0][:])
                nc.sync.dma_start(dbg_t["dast"][:, B:2 * B], asts[1][:])
                nc.sync.dma_start(dbg_t["dls"][:, 0:B], lss[0][:])
                nc.sync.dma_start(dbg_t["dls"][:, B:2 * B], lss[1][:])

    nc.compile()
    return nc


# ---------------------------------------------------------------------------
# host side
# ---------------------------------------------------------------------------

def _pack_dir(Wx, Wh, b):
    i, f, g, o = np.split(np.asarray(Wx, np.float32), 4, axis=1)
    wxp = np.concatenate([2.0 * g, i, f, o], axis=1)
    i, f, g, o = np.split(np.asarray(Wh, np.float32), 4, axis=1)
    whp = np.concatenate([4.0 * g, 2.0 * i, 2.0 * f, 2.0 * o], axis=1)
    bi, bf_, bg, bo = np.split(np.asarray(b, np.float32), 4)
    bp = np.concatenate([2.0 * bg, bi, bf_, bo])
    return np.concatenate([wxp, bp[None, :]], axis=0), whp


def make_in_maps(inputs, labels, E, Wx_f, Wh_f, b_f, Wx_b, Wh_b, b_b, Wd, bd, T,
                 L=L_FULL):
    NT = B * L
    bf16 = ml_dtypes.bfloat16
    wxf, whf = _pack_dir(Wx_f, Wh_f, b_f)
    wxb, whb = _pack_dir(Wx_b, Wh_b, b_b)
    wx = np.concatenate([wxf, wxb], axis=1).astype(bf16)
    wh = np.concatenate([whf, whb], axis=1).astype(bf16)
    wd = (2.0 * np.asarray(Wd, np.float32)).astype(bf16)
    bdt = np.asarray(bd, np.float32).reshape(K, 1)
    tmat = np.asarray(T, np.float32)
    emb = np.ascontiguousarray(np.asarray(E, np.float32))
    tok = np.asarray(inputs).astype(np.int32)
    lab = np.asarray(labels).astype(np.int32)

    in_maps = []
    for c in range(NCORES):
        tk = tok[c * B:(c + 1) * B]          # [B, L]
        lb = lab[c * B:(c + 1) * B]
        ids = np.ascontiguousarray(tk.T).reshape(-1)          # t-major [NT]
        xidx = np.ascontiguousarray(ids.reshape(NT // 128, 128).T).astype(np.int32)
        labt = np.ascontiguousarray(lb.T).reshape(-1)
        oh = (labt[None, :] == np.arange(K, dtype=np.int64)[:, None])
        lens = (lb != 0).sum(axis=1)                          # [B]
        apf = (np.arange(L)[None, :] >= lens[:, None])        # [B, L] invalid
        apt = np.ascontiguousarray(apf.T).reshape(-1).astype(np.float32)
        pen = (-10000.0 * apt)[None, :]
        pen[0, 0:B] = 0.0
        pen = pen.astype(bf16)
        in_maps.append(dict(
            emb=emb, xidx=xidx, wx=wx, wh=wh, wd=wd, bdt=bdt, tmat=tmat,
            ohd=oh.astype(bf16), pend=pen, grvd=apt[None, :].astype(bf16),
        ))
    return in_maps


_PROG = None


def _get_prog():
    global _PROG
    if _PROG is None:
        _PROG = build_program()
    return _PROG


def kernel(inputs, labels, E, Wx_f, Wh_f, b_f, Wx_b, Wh_b, b_b, Wd, bd, T):
    nc = _get_prog()
    in_maps = make_in_maps(inputs, labels, E, Wx_f, Wh_f, b_f,
                           Wx_b, Wh_b, b_b, Wd, bd, T)
    res = run_bass_kernel_spmd(nc, in_maps, core_ids=list(range(NCORES)))
    ll = np.concatenate([res.results[c]["ll"].reshape(B) for c in range(NCORES)])
    return ll.astype(np.float32), np.asarray(T, np.float32)


# numpy mini-reference (float64) for testing at arbitrary L/V ----------------

def ref_numpy(inputs, labels, E, Wx_f, Wh_f, b_f, Wx_b, Wh_b, b_b, Wd, bd, T):
    f = np.float64
    tok = np.asarray(inputs); lab = np.asarray(labels)
    E = np.asarray(E, f); T = np.asarray(T, f)
    Bf, Lf = tok.shape

    def sig(x):
        return 1.0 / (1.0 + np.exp(-x))

    def lstm(x, Wx, Wh, b, reverse):
        Wx = np.asarray(Wx, f); Wh = np.asarray(Wh, f); b = np.asarray(b, f)
        h = np.zeros((Bf, RNN), f); c = np.zeros((Bf, RNN), f)
        hs = np.zeros((Lf, Bf, RNN), f)
        order = range(Lf - 1, -1, -1) if reverse else range(Lf)
        for t in order:
            z = x[t] @ Wx + h @ Wh + b
            i, fg, g, o = np.split(z, 4, axis=1)
            c = sig(fg) * c + sig(i) * np.tanh(g)
            h = sig(o) * np.tanh(c)
            hs[t] = h
        return hs

    x = E[tok].transpose(1, 0, 2)             # [L, B, E]
    hf = lstm(x, Wx_f, Wh_f, b_f, False)
    hb = lstm(x, Wx_b, Wh_b, b_b, True)
    h = np.concatenate([hf, hb], axis=2)      # [L, B, 2R]
    logits = h.transpose(1, 0, 2) @ np.asarray(Wd, f) + np.asarray(bd, f)
    lens = (lab != 0).sum(axis=1)
    pos = np.arange(Lf)[None, :] < lens[:, None]
    unary = np.take_along_axis(logits, lab[..., None], axis=2)[..., 0]
    unary = (unary * pos).sum(axis=1)
    binary = (T[lab[:, :-1], lab[:, 1:]] * pos[:, 1:]).sum(axis=1)
    alpha = logits[:, 0, :].copy()
    for t in range(1, Lf):
        new = np.log(np.exp(alpha[:, :, None] - alpha.max(1)[:, None, None]
                            + T[None]).sum(axis=1)) \
            + alpha.max(1)[:, None] + logits[:, t, :]
        alpha = np.where(pos[:, t][:, None], new, alpha)
    mx = alpha.max(1)
    log_norm = np.log(np.exp(alpha - mx[:, None]).sum(1)) + mx
    return unary + binary - log_norm


# revision 31
# speedup vs baseline: 1.0215x; 1.0034x over previous
"""BiLSTM-CRF loss on 8 Trainium2 NeuronCores (pure data parallel over batch).

Strategy (per core, batch shard B=64):
  Phase 0: embedding gather (indirect DMA row gather) + PE transpose -> xT [65, B*L]
           (row 64 = ones, so the LSTM bias rides the x-matmul).
  Loop 1:  fwd and bwd LSTM as two independent chains that pipeline against
           each other.  All four gates use one sigmoid per chain
           (tanh(x) = 2*sigmoid(2x)-1 folded into the weights; hidden state is
           tracked as h' = h/2 so h' = (sigmoid(2c)-0.5)*sigmoid(z_o)); the
           cell c is transposed on the PE and sigmoid(2c^T) reads the PSUM
           directly, writing the transposed bf16 state straight into the
           per-step hidden store used by the logits pass.
  Loop 2:  logits^T chunks [18, 512] = Wd'^T hT (+ -1e4 * invalid-mask via a
           K=1 matmul) -> exp(logits+bd) -> g;  ghat = g * onehot(labels).
  Loop 3:  CRF forward recurrence + gold-path score as two parallel scaled
           exp-domain chains: state [19, 128] (19th row = "graveyard" that
           captures the terminal mass when a sequence's mask ends), one 19x19
           constant-stationary matmul + one DVE multiply per step, periodic
           per-column rescaling (every 12 steps: keeps within-window growth
           under fp32 range AND the final pre-Ln colsum under the ScalarE Ln
           domain limit of 2^64) with log-scale accumulation.
  ll = path_score - log_norm, gathered to the host.
"""

import numpy as np
import ml_dtypes
from contextlib import ExitStack

import concourse.bacc as bacc
import concourse.bass as bass
import concourse.tile as tile
from concourse import mybir
from concourse.bass_utils import run_bass_kernel_spmd
from concourse.masks import make_identity

AF = mybir.ActivationFunctionType
ALU = mybir.AluOpType
F32 = mybir.dt.float32
BF16 = mybir.dt.bfloat16
I32 = mybir.dt.int32

EMB = 64
RNN = 128
K = 18
NCORES = 8
B = 64          # batch rows per core
L_FULL = 256
V_FULL = 30001
G = 4 * RNN     # 512 gate columns per direction


def build_program(L=L_FULL, V=V_FULL, w_rescale=12, dbg=False, stop_after="full"):
    NT = B * L
    nc = bacc.Bacc("TRN2", target_bir_lowering=False, debug=False)
    dbg_t = {}
    if dbg:
        dbg_t["dxT"] = nc.dram_tensor("dxT", [EMB + 1, NT], BF16, kind="ExternalOutput")
        dbg_t["dhj"] = nc.dram_tensor("dhj", [128, L * 128], BF16, kind="ExternalOutput")
        dbg_t["dgg"] = nc.dram_tensor("dgg", [K + 1, 2 * NT], BF16, kind="ExternalOutput")
        dbg_t["dast"] = nc.dram_tensor("dast", [K + 1, 2 * B], F32, kind="ExternalOutput")
        dbg_t["dls"] = nc.dram_tensor("dls", [1, 2 * B], F32, kind="ExternalOutput")
        dbg_t["dr1"] = nc.dram_tensor("dr1", [1, 2 * B], F32, kind="ExternalOutput")
        dbg_t["dz1"] = nc.dram_tensor("dz1", [1, 2 * B], F32, kind="ExternalOutput")
        dbg_t["da13"] = nc.dram_tensor("da13", [K + 1, 2 * B], F32, kind="ExternalOutput")

    emb = nc.dram_tensor("emb", [V, EMB], F32, kind="ExternalInput")
    xidx = nc.dram_tensor("xidx", [128, NT // 128], I32, kind="ExternalInput")
    wx = nc.dram_tensor("wx", [EMB + 1, 2 * G], BF16, kind="ExternalInput")
    wh = nc.dram_tensor("wh", [RNN, 2 * G], BF16, kind="ExternalInput")
    wd = nc.dram_tensor("wd", [2 * RNN, K], BF16, kind="ExternalInput")
    bdt = nc.dram_tensor("bdt", [K, 1], F32, kind="ExternalInput")
    tmat = nc.dram_tensor("tmat", [K, K], F32, kind="ExternalInput")
    ohd = nc.dram_tensor("ohd", [K, NT], BF16, kind="ExternalInput")
    pend = nc.dram_tensor("pend", [1, NT], BF16, kind="ExternalInput")
    grvd = nc.dram_tensor("grvd", [1, NT], BF16, kind="ExternalInput")
    llo = nc.dram_tensor("ll", [1, B], F32, kind="ExternalOutput")

    with tile.TileContext(nc) as tc, ExitStack() as ctx:
        const = ctx.enter_context(tc.tile_pool(name="const", bufs=1))
        persist = ctx.enter_context(tc.tile_pool(name="persist", bufs=1))

        ident = const.tile([128, 128], F32)
        make_identity(nc, ident[:])
        ident_bf = const.tile([128, 128], BF16)
        nc.vector.tensor_copy(ident_bf[:], ident[:])
        wx_sb = const.tile([EMB + 1, 2 * G], BF16)
        nc.sync.dma_start(wx_sb[:], wx[:])
        wh_sb = const.tile([RNN, 2 * G], BF16)
        nc.sync.dma_start(wh_sb[:], wh[:])
        wdf_sb = const.tile([RNN, K], BF16)
        nc.sync.dma_start(wdf_sb[:], wd[0:RNN, :])
        wdb_sb = const.tile([RNN, K], BF16)
        nc.sync.dma_start(wdb_sb[:], wd[RNN:2 * RNN, :])
        bdt_sb = const.tile([K, 1], F32)
        nc.sync.dma_start(bdt_sb[:], bdt[:])
        ones18 = const.tile([1, K], BF16)
        nc.vector.memset(ones18[:], 1.0)
        ones19 = const.tile([1, K + 1], F32)
        nc.vector.memset(ones19[:], 1.0)

        # Maug [19,33]: cols 0..17 = exp(T), col 18 = grave (ones: live-sum +
        # grave passthrough), col 32 = colsum readout (ones) placed at
        # partition 32 of the matmul output so PSUM row reads are aligned.
        MS = 33
        tm_sb = const.tile([K, K], F32)
        nc.sync.dma_start(tm_sb[:], tmat[:])
        maug = const.tile([K + 1, MS], F32)
        nc.vector.memset(maug[:], 0.0)
        nc.scalar.activation(maug[0:K, 0:K], tm_sb[:], AF.Exp)
        nc.vector.memset(maug[:, K:K + 1], 1.0)
        nc.vector.memset(maug[:, MS - 1:MS], 1.0)

        # persistent store: col t*128+b -> fwd hT for b<64, bwd hT for 64+b
        hjoint = persist.tile([128, L * 128], BF16)

        # ---------------- Phase 0 + Loop 1 (xT lives only here) -------------
        with tc.tile_pool(name="xT", bufs=1) as xpool:
            xT = xpool.tile([EMB + 1, NT], BF16)
            nc.vector.memset(xT[EMB:EMB + 1, :], 1.0)
            with (
                tc.tile_pool(name="gath", bufs=6) as gpool0,
                tc.tile_pool(name="idx", bufs=1) as ipool,
                tc.tile_pool(name="state", bufs=2) as spool,
                tc.tile_pool(name="zps", bufs=2, space="PSUM") as zpool,
                tc.tile_pool(name="tps", bufs=4, space="PSUM") as tpool,
                tc.tile_pool(name="gates", bufs=3) as gpool,
            ):
                idxt = ipool.tile([128, NT // 128], I32)
                nc.sync.dma_start(idxt[:], xidx[:])
                NK = NT // 128

                def emit_xtile(k):
                    xg = gpool0.tile([128, EMB], F32, tag="xg")
                    nc.gpsimd.indirect_dma_start(
                        out=xg[:],
                        out_offset=None,
                        in_=emb[:],
                        in_offset=bass.IndirectOffsetOnAxis(
                            ap=idxt[:, k:k + 1], axis=0),
                    )
                    xps = tpool.tile([EMB, 128], F32, tag="tps")
                    nc.tensor.transpose(xps[:], xg[:], ident[:])
                    nc.vector.tensor_copy(
                        xT[0:EMB, k * 128:(k + 1) * 128], xps[:])

                def emit_pair(m):  # gather tiles first needed at iter m (even)
                    emit_xtile(m // 2)
                    if NK - 1 - m // 2 != m // 2:
                        emit_xtile(NK - 1 - m // 2)

                PD = 6
                for m in range(0, PD, 2):
                    if m <= 2 * (NK - 1):
                        emit_pair(m)
                # two independent chains: F(orward) and B(ackward), each B=64,
                # gate column order [g, i, f, o]
                hz = spool.tile([128, 2 * B], BF16, tag="h0")
                nc.vector.memset(hz[:], 0.0)
                h_prev = [hz[:, 0:B], hz[:, B:2 * B]]
                c_prev = []
                for ci in range(2):
                    cz = spool.tile([B, RNN], F32, tag=f"c{ci}")
                    nc.vector.memset(cz[:], 0.0)
                    c_prev.append(cz)
                for i in range(L):
                    if (i + PD) % 2 == 0 and i + PD <= 2 * (NK - 1):
                        emit_pair(i + PD)
                    ts_ = (i, L - 1 - i)
                    zs = []
                    for ci in range(2):
                        t = ts_[ci]
                        z = zpool.tile([B, G], F32, tag=f"z{ci}")
                        nc.tensor.matmul(z[:], lhsT=xT[:, t * B:(t + 1) * B],
                                         rhs=wx_sb[:, ci * G:(ci + 1) * G],
                                         start=True, stop=False)
                        nc.tensor.matmul(z[:], lhsT=h_prev[ci],
                                         rhs=wh_sb[:, ci * G:(ci + 1) * G],
                                         start=False, stop=True)
                        zs.append(z)
                    ss = []
                    for ci in range(2):
                        s = gpool.tile([B, G], BF16, tag=f"s{ci}")
                        nc.scalar.activation(s[:], zs[ci][:], AF.Sigmoid)
                        ss.append(s)
                    cs = []
                    for ci in range(2):
                        s = ss[ci]
                        v = gpool.tile([B, RNN], BF16, tag=f"v{ci}")
                        nc.vector.scalar_tensor_tensor(
                            v[:], in0=s[:, 0:RNN], scalar=-0.5,
                            in1=s[:, RNN:2 * RNN], op0=ALU.add, op1=ALU.mult)
                        p_ = gpool.tile([B, RNN], F32, tag=f"p{ci}")
                        nc.vector.tensor_tensor(p_[:], s[:, 2 * RNN:3 * RNN],
                                                c_prev[ci][:], op=ALU.mult)
                        c_new = spool.tile([B, RNN], F32, tag=f"c{ci}")
                        nc.vector.scalar_tensor_tensor(
                            c_new[:], in0=v[:], scalar=2.0, in1=p_[:],
                            op0=ALU.mult, op1=ALU.add)
                        cs.append(c_new)
                    ots = []
                    for ci in range(2):
                        ot_ps = tpool.tile([128, B], BF16, tag="tps")
                        nc.tensor.transpose(ot_ps[:], ss[ci][:, 3 * RNN:4 * RNN],
                                            ident_bf[0:B, 0:B])
                        ot = gpool.tile([128, B], BF16, tag=f"ot{ci}")
                        nc.vector.tensor_copy(ot[:], ot_ps[:])
                        ots.append(ot)
                    scts = []
                    for ci in range(2):
                        ct_ps = tpool.tile([128, B], F32, tag="tps")
                        nc.tensor.transpose(ct_ps[:], cs[ci][:], ident[0:B, 0:B])
                        sct = gpool.tile([128, B], BF16, tag=f"sct{ci}")
                        nc.scalar.activation(sct[:], ct_ps[:], AF.Sigmoid,
                                             scale=2.0)
                        scts.append(sct)
                    newh = []
                    for ci in range(2):
                        t = ts_[ci]
                        hsl = hjoint[:, t * 128 + ci * B:t * 128 + (ci + 1) * B]
                        nc.vector.scalar_tensor_tensor(
                            hsl, in0=scts[ci][:], scalar=-0.5, in1=ots[ci][:],
                            op0=ALU.add, op1=ALU.mult)
                        newh.append(hsl)
                    h_prev = newh
                    c_prev = cs
        if dbg:
            nc.sync.dma_start(dbg_t["dhj"][:], hjoint[:])
        if stop_after == "l1":
            nc.sync.dma_start(llo[:], hjoint[0:1, 0:2 * B].bitcast(F32))
        # ---------------- Loop 2: logits -> g, ghat ------------------------
        TCH = 8
        ggpool = ctx.enter_context(tc.tile_pool(name="gg", bufs=1))
        if stop_after == "l1":
            gg = None
        else:
            gg = ggpool.tile([K + 1, 2 * NT], BF16)   # [alpha-g | path-ghat]
            nc.sync.dma_start(gg[K:K + 1, 0:NT], grvd[:])
            nc.sync.dma_start(gg[K:K + 1, NT:2 * NT], grvd[:])
        do_l3 = stop_after == "full"
        with (
            tc.tile_pool(name="l2in", bufs=3) as l2pool,
            tc.tile_pool(name="lps", bufs=2, space="PSUM") as lpool,
            tc.tile_pool(name="crf", bufs=3) as cpool,
            tc.tile_pool(name="crfps", bufs=2, space="PSUM") as cps,
        ):
            hj3 = hjoint[:].rearrange("p (t c) -> p t c", c=128)

            def emit_chunk(q):
                t0 = q * TCH
                cw = TCH * B
                ohc = l2pool.tile([K, cw], BF16, tag="ohc")
                nc.sync.dma_start(ohc[:], ohd[:, t0 * B:t0 * B + cw])
                penc = l2pool.tile([1, cw], BF16, tag="penc")
                nc.sync.dma_start(penc[:], pend[:, t0 * B:t0 * B + cw])
                lp = lpool.tile([K, cw], F32)
                nc.tensor.matmul(lp[:], lhsT=wdf_sb[:],
                                 rhs=hj3[:, t0:t0 + TCH, 0:B],
                                 start=True, stop=False)
                nc.tensor.matmul(lp[:], lhsT=wdb_sb[:],
                                 rhs=hj3[:, t0:t0 + TCH, B:2 * B],
                                 start=False, stop=False)
                nc.tensor.matmul(lp[:], lhsT=ones18[:], rhs=penc[:],
                                 start=False, stop=True)
                nc.scalar.activation(gg[0:K, t0 * B:t0 * B + cw], lp[:],
                                     AF.Exp, bias=bdt_sb[:])
                nc.vector.tensor_tensor(
                    gg[0:K, NT + t0 * B:NT + t0 * B + cw],
                    gg[0:K, t0 * B:t0 * B + cw],
                    ohc[:], op=ALU.mult)

            NQ = L // TCH
            if not do_l3:
                for q in range(NQ if stop_after != "l1" else 0):
                    emit_chunk(q)
            else:
                for q in range(min(3, NQ)):
                    emit_chunk(q)
                asts = []
                for ci in range(2):
                    a0_ = cpool.tile([K + 1, B], F32, tag=f"ast{ci}")
                    nc.vector.memset(a0_[:], 0.0)
                    nc.vector.tensor_copy(a0_[0:K, :],
                                          gg[0:K, ci * NT:ci * NT + B])
                    asts.append(a0_)
                lss = []
                for ci in range(2):
                    l0 = cpool.tile([1, B], F32, tag=f"ls{ci}")
                    nc.vector.memset(l0[:], 0.0)
                    lss.append(l0)
                for t in range(1, L):
                    if t % TCH == 0 and t // TCH + 2 < NQ:
                        emit_chunk(t // TCH + 2)
                    pas = []
                    for ci in range(2):
                        pa = cps.tile([MS, B], F32, tag=f"pa{ci}")
                        nc.tensor.matmul(pa[:], lhsT=maug[:], rhs=asts[ci][:],
                                         start=True, stop=True)
                        pas.append(pa)
                    resc = (t % w_rescale == 0)
                    news = []
                    for ci in range(2):
                        a_new = cpool.tile([K + 1, B], F32, tag=f"ast{ci}")
                        gsl = gg[0:K + 1,
                                 ci * NT + t * B:ci * NT + (t + 1) * B]
                        if resc:
                            zrow = cpool.tile([1, B], F32, tag=f"zr{ci}")
                            nc.vector.tensor_copy(zrow[:], pas[ci][MS - 1:MS, :])
                            r = cpool.tile([1, B], F32, tag=f"r{ci}")
                            nc.vector.reciprocal(r[:], zrow[:])
                            pr = cps.tile([K + 1, B], F32, tag="pr")
                            nc.tensor.matmul(pr[:], lhsT=ones19[:], rhs=r[:],
                                             start=True, stop=True)
                            lnr = cpool.tile([1, B], F32, tag=f"lnr{ci}")
                            nc.scalar.activation(lnr[:], r[:], AF.Ln)
                            ls_new = cpool.tile([1, B], F32, tag=f"ls{ci}")
                            nc.vector.tensor_tensor(ls_new[:], lss[ci][:],
                                                    lnr[:], op=ALU.subtract)
                            lss[ci] = ls_new
                            atmp = cpool.tile([K + 1, B], F32, tag=f"atmp{ci}")
                            nc.vector.tensor_tensor(atmp[:], pas[ci][0:K + 1, :],
                                                    gsl, op=ALU.mult)
                            nc.vector.tensor_tensor(a_new[:], atmp[:], pr[:],
                                                    op=ALU.mult)
                        else:
                            nc.vector.tensor_tensor(a_new[:],
                                                    pas[ci][0:K + 1, :],
                                                    gsl, op=ALU.mult)
                        news.append(a_new)
                    asts = news
                tots = []
                for ci in range(2):
                    pf = cps.tile([MS, B], F32, tag=f"pa{ci}")
                    nc.tensor.matmul(pf[:], lhsT=maug[:], rhs=asts[ci][:],
                                     start=True, stop=True)
                    lnt = cpool.tile([1, B], F32, tag=f"lnt{ci}")
                    nc.scalar.activation(lnt[:], pf[MS - 1:MS, :], AF.Ln)
                    tot = cpool.tile([1, B], F32, tag=f"tot{ci}")
                    nc.vector.tensor_tensor(tot[:], lnt[:], lss[ci][:],
                                            op=ALU.add)
                    tots.append(tot)
                ll_sb = cpool.tile([1, B], F32, tag="ll")
                nc.vector.tensor_tensor(ll_sb[:], tots[1][:], tots[0][:],
                                        op=ALU.subtract)
                nc.sync.dma_start(llo[:], ll_sb[:])
                if dbg:
                    nc.sync.dma_start(dbg_t["dast"][:, 0:B], asts[0][:])
                    nc.sync.dma_start(dbg_t["dast"][:, B:2 * B], asts[1][:])
                    nc.sync.dma_start(dbg_t["dls"][:, 0:B], lss[0][:])
                    nc.sync.dma_start(dbg_t["dls"][:, B:2 * B], lss[1][:])
        if stop_after == "l2":
            nc.sync.dma_start(llo[:], gg[0:1, 0:2 * B].bitcast(F32))

    nc.compile()
    return nc


# ---------------------------------------------------------------------------
# host side
# ---------------------------------------------------------------------------

def _pack_dir(Wx, Wh, b):
    i, f, g, o = np.split(np.asarray(Wx, np.float32), 4, axis=1)
    wxp = np.concatenate([2.0 * g, i, f, o], axis=1)
    i, f, g, o = np.split(np.asarray(Wh, np.float32), 4, axis=1)
    whp = np.concatenate([4.0 * g, 2.0 * i, 2.0 * f, 2.0 * o], axis=1)
    bi, bf_, bg, bo = np.split(np.asarray(b, np.float32), 4)
    bp = np.concatenate([2.0 * bg, bi, bf_, bo])
    return np.concatenate([wxp, bp[None, :]], axis=0), whp


def make_in_maps(inputs, labels, E, Wx_f, Wh_f, b_f, Wx_b, Wh_b, b_b, Wd, bd, T,
                 L=L_FULL):
    NT = B * L
    bf16 = ml_dtypes.bfloat16
    wxf, whf = _pack_dir(Wx_f, Wh_f, b_f)
    wxb, whb = _pack_dir(Wx_b, Wh_b, b_b)
    wx = np.concatenate([wxf, wxb], axis=1).astype(bf16)
    wh = np.concatenate([whf, whb], axis=1).astype(bf16)
    wd = (2.0 * np.asarray(Wd, np.float32)).astype(bf16)
    bdt = np.asarray(bd, np.float32).reshape(K, 1)
    tmat = np.asarray(T, np.float32)
    emb = np.ascontiguousarray(np.asarray(E, np.float32))
    tok = np.asarray(inputs).astype(np.int32)
    lab = np.asarray(labels).astype(np.int32)

    in_maps = []
    for c in range(NCORES):
        tk = tok[c * B:(c + 1) * B]          # [B, L]
        lb = lab[c * B:(c + 1) * B]
        ids = np.ascontiguousarray(tk.T).reshape(-1)          # t-major [NT]
        xidx = np.ascontiguousarray(ids.reshape(NT // 128, 128).T).astype(np.int32)
        labt = np.ascontiguousarray(lb.T).reshape(-1)
        oh = (labt[None, :] == np.arange(K, dtype=np.int64)[:, None])
        lens = (lb != 0).sum(axis=1)                          # [B]
        apf = (np.arange(L)[None, :] >= lens[:, None])        # [B, L] invalid
        apt = np.ascontiguousarray(apf.T).reshape(-1).astype(np.float32)
        pen = (-10000.0 * apt)[None, :]
        pen[0, 0:B] = 0.0
        pen = pen.astype(bf16)
        in_maps.append(dict(
            emb=emb, xidx=xidx, wx=wx, wh=wh, wd=wd, bdt=bdt, tmat=tmat,
            ohd=oh.astype(bf16), pend=pen, grvd=apt[None, :].astype(bf16),
        ))
    return in_maps


_PROG = None


def _get_prog():
    global _PROG
    if _PROG is None:
        _PROG = build_program()
    return _PROG


def kernel(inputs, labels, E, Wx_f, Wh_f, b_f, Wx_b, Wh_b, b_b, Wd, bd, T):
    nc = _get_prog()
    in_maps = make_in_maps(inputs, labels, E, Wx_f, Wh_f, b_f,
                           Wx_b, Wh_b, b_b, Wd, bd, T)
    res = run_bass_kernel_spmd(nc, in_maps, core_ids=list(range(NCORES)))
    ll = np.concatenate([res.results[c]["ll"].reshape(B) for c in range(NCORES)])
    return ll.astype(np.float32), np.asarray(T, np.float32)


# numpy mini-reference (float64) for testing at arbitrary L/V ----------------

def ref_numpy(inputs, labels, E, Wx_f, Wh_f, b_f, Wx_b, Wh_b, b_b, Wd, bd, T):
    f = np.float64
    tok = np.asarray(inputs); lab = np.asarray(labels)
    E = np.asarray(E, f); T = np.asarray(T, f)
    Bf, Lf = tok.shape

    def sig(x):
        return 1.0 / (1.0 + np.exp(-x))

    def lstm(x, Wx, Wh, b, reverse):
        Wx = np.asarray(Wx, f); Wh = np.asarray(Wh, f); b = np.asarray(b, f)
        h = np.zeros((Bf, RNN), f); c = np.zeros((Bf, RNN), f)
        hs = np.zeros((Lf, Bf, RNN), f)
        order = range(Lf - 1, -1, -1) if reverse else range(Lf)
        for t in order:
            z = x[t] @ Wx + h @ Wh + b
            i, fg, g, o = np.split(z, 4, axis=1)
            c = sig(fg) * c + sig(i) * np.tanh(g)
            h = sig(o) * np.tanh(c)
            hs[t] = h
        return hs

    x = E[tok].transpose(1, 0, 2)             # [L, B, E]
    hf = lstm(x, Wx_f, Wh_f, b_f, False)
    hb = lstm(x, Wx_b, Wh_b, b_b, True)
    h = np.concatenate([hf, hb], axis=2)      # [L, B, 2R]
    logits = h.transpose(1, 0, 2) @ np.asarray(Wd, f) + np.asarray(bd, f)
    lens = (lab != 0).sum(axis=1)
    pos = np.arange(Lf)[None, :] < lens[:, None]
    unary = np.take_along_axis(logits, lab[..., None], axis=2)[..., 0]
    unary = (unary * pos).sum(axis=1)
    binary = (T[lab[:, :-1], lab[:, 1:]] * pos[:, 1:]).sum(axis=1)
    alpha = logits[:, 0, :].copy()
    for t in range(1, Lf):
        new = np.log(np.exp(alpha[:, :, None] - alpha.max(1)[:, None, None]
                            + T[None]).sum(axis=1)) \
            + alpha.max(1)[:, None] + logits[:, t, :]
        alpha = np.where(pos[:, t][:, None], new, alpha)
    mx = alpha.max(1)
    log_norm = np.log(np.exp(alpha - mx[:, None]).sum(1)) + mx
    return unary + binary - log_norm
